# revision 12
# baseline (speedup 1.0000x reference)
"""Expert-parallel grouped-MLP (MoE experts) kernel for 8 Trainium2 cores.

Problem: y = W2_e @ silu(W1_e @ x_e + b1_e) + b2_e for E=16 independent
experts (grouped 1x1 conv), B=8 batches, C=256 channels/expert, CAP=4,
L=1024 positions.

Sharding: expert-parallel — core i owns experts {2i, 2i+1}; no cross-core
communication. All operands are cast/packed to fp16 lhsT layouts on the
HOST, so the device does zero dtype conversion and every load is a single
contiguous DMA:

  per (b, e) pair:  1 DMA x[128, 2048] fp16 (both k-tiles merged)
    layer 1: 8 m-tiles x (2k x 2n) fp16 matmuls -> PSUM[128,1024] fp32
             ACT silu(. + b1) PSUM -> h SBUF [128, 8x1024] fp16
    layer 2: 2 j-tiles x (8q x 2n) accumulating matmuls -> PSUM[128,1024]
             DVE + b2 PSUM -> y SBUF fp16, 1 merged DMA out per pair

Output is fp16 on device, upcast to fp32 on the host. PE floor is
1024 matmuls x 512 cols @ 2.4GHz = 218.4us/core; startup (fp16 weights,
no casts) and tail (fewer DMAs/semaphores) are trimmed vs the fp32-staged
version. Warmup matmuls cover the PE p-state ramp during initial DMAs.
"""
import numpy as np

import concourse.tile as tile
from concourse import bacc, mybir
from concourse.bass_utils import run_bass_kernel_spmd

# Problem constants (hardcoded per contract)
B, E, C, CAP, L = 8, 16, 256, 4, 1024
F = C * CAP            # 1024 hidden per expert
NCORES = 8
EPC = E // NCORES      # 2 experts per core
P = 128                # partitions
KT = C // P            # 2 k-tiles (layer-1 contraction)
MT = F // P            # 8 m-tiles (layer-1 output partitions)
JT = C // P            # 2 j-tiles (layer-2 output partitions)
QT = F // P            # 8 q-tiles (layer-2 contraction)
NT = L // 512          # 2 n-tiles of 512 cols
N_WARMUP = 5           # dummy PE warmup matmuls (cover p-state ramp)

_FP32 = mybir.dt.float32
_FP16 = mybir.dt.float16


def _build():
    nc = bacc.Bacc("TRN2", target_bir_lowering=False, debug=False)

    xs_d = nc.dram_tensor("xs", [B, EPC * C, L], _FP16, kind="ExternalInput")
    w1p_d = nc.dram_tensor("w1p", [EPC * KT * P, F], _FP16, kind="ExternalInput")
    b1p_d = nc.dram_tensor("b1p", [EPC * P, MT], _FP32, kind="ExternalInput")
    w2p_d = nc.dram_tensor("w2p", [EPC * P, QT * C], _FP16, kind="ExternalInput")
    b2p_d = nc.dram_tensor("b2p", [EPC * P, JT], _FP32, kind="ExternalInput")
    ys_d = nc.dram_tensor("ys", [B, EPC * C, L], _FP16, kind="ExternalOutput")

    with tile.TileContext(nc) as tc:
        with (
            tc.tile_pool(name="const", bufs=1) as cpool,
            tc.tile_pool(name="x", bufs=3) as xpool,
            tc.tile_pool(name="h", bufs=2) as hpool,
            tc.tile_pool(name="y", bufs=2) as ypool,
            tc.tile_pool(name="ps", bufs=4, space="PSUM") as pspool,
        ):
            # ---- PE warmup: zero bf16 matmuls with no DMA deps ----
            wdum = cpool.tile([P, P], mybir.dt.bfloat16, tag="wdum")
            rdum = cpool.tile([P, 512], mybir.dt.bfloat16, tag="rdum")
            # memset on Pool: its post-barrier slot opens earliest, so the
            # first warmup matmul issues ~0.6us sooner than via DVE
            nc.gpsimd.memset(wdum[:], 0.0)
            nc.gpsimd.memset(rdum[:], 0.0)
            for i in range(N_WARMUP):
                pdum = pspool.tile([P, L], _FP32, tag="ps")
                nc.tensor.matmul(pdum[:, :512], wdum[:], rdum[:],
                                 start=True, stop=True)

            # ---- weight/bias tiles: host-packed fp16, single DMAs ----
            # w1sb[e][k]: [128, F];       [p, f] = W1r[ge, f, k*128+p]
            # w2sb[e]:    [128, QT*C];    [p, q*C+c] = W2r[ge, c, q*128+p]
            w1sb = [[cpool.tile([P, F], _FP16, tag=f"w1_{e}_{k}",
                                name=f"w1sb_{e}_{k}")
                     for k in range(KT)] for e in range(EPC)]
            w2sb = [cpool.tile([P, QT * C], _FP16, tag=f"w2_{e}",
                               name=f"w2sb_{e}")
                    for e in range(EPC)]
            b1sb = cpool.tile([P, EPC * MT], _FP32, tag="b1")  # col e*MT+m
            b2sb = cpool.tile([P, EPC * JT], _FP32, tag="b2")  # col e*JT+j

            # weight/bias loads go on the Activation engine's HW-DGE so
            # their transfers run on a separate hardware queue from the
            # x/y stream on the SP queue (each DGE serializes its queue)
            def load_w1(e, k, eng):
                eng.dma_start(
                    w1sb[e][k][:],
                    w1p_d.ap()[(e * KT + k) * P:(e * KT + k + 1) * P, :],
                )

            def load_w2(e, eng):
                eng.dma_start(
                    w2sb[e][:],
                    w2p_d.ap()[e * P:(e + 1) * P, :],
                )

            def load_b1(e, eng):
                eng.dma_start(
                    b1sb[:, e * MT:(e + 1) * MT],
                    b1p_d.ap()[e * P:(e + 1) * P, :],
                )

            def load_b2(e, eng):
                eng.dma_start(
                    b2sb[:, e * JT:(e + 1) * JT],
                    b2p_d.ap()[e * P:(e + 1) * P, :],
                )

            def load_x(b, e, split=False):
                # one tile [128, KT*L] fp16; col k*L+l = x[b, e*C+k*128+p, l]
                xt = xpool.tile([P, KT * L], _FP16, tag="x", name=f"x_{b}_{e}")
                if split:
                    # first pair: per-k DMAs on BOTH DGE queues so the two
                    # halves transfer in parallel with the w1 k-tiles
                    for k, eng in ((0, nc.sync), (1, nc.scalar)):
                        eng.dma_start(
                            xt[:, k * L:(k + 1) * L],
                            xs_d.ap()[b, e * C + k * P:e * C + (k + 1) * P, :],
                        )
                else:
                    nc.sync.dma_start(
                        xt[:].rearrange("p (k l) -> p k l", k=KT),
                        xs_d.ap()[b, e * C:(e + 1) * C, :]
                        .rearrange("(k p) l -> p k l", p=P),
                    )
                return xt

            # startup-critical order, two parallel queues:
            #   SP DGE:  w1(0,0), x0[k0], x(pair1), ...
            #   ACT DGE: w1(0,1), x0[k1], b1, b2, w2 (needed ~20us in)
            # expert-1 weights are deferred into the pair loop (needed
            # only at pair 8).
            load_w1(0, 0, nc.sync)
            load_w1(0, 1, nc.scalar)
            x0 = load_x(0, 0, split=True)
            load_b1(0, nc.scalar)
            load_b2(0, nc.scalar)
            load_w2(0, nc.scalar)
            # actdum AFTER the ACT-DGE triggers: the lazy ACT_TABLE_LOAD
            # (~1.3us) it induces must not delay the startup weight DMAs
            actdum = cpool.tile([P, 1], _FP32, tag="actdum")
            nc.scalar.activation(actdum[:], rdum[:, :1],
                                 mybir.ActivationFunctionType.Silu, bias=0.0)

            pairs = [(e, b) for e in range(EPC) for b in range(B)]
            xtiles = {(0, 0): x0}
            xtiles[pairs[1]] = load_x(pairs[1][1], pairs[1][0])

            # ---- per-(expert, batch) pipeline ----
            for t, (e, b) in enumerate(pairs):
                xsb = xtiles.pop((e, b))
                if t + 2 < len(pairs):
                    pe, pb = pairs[t + 2]
                    xtiles[(pe, pb)] = load_x(pb, pe)
                if t == 1:
                    # early prefetches in flight; queue expert-1 weights on
                    # the ACT DGE (needed at pair 8, ~120us in)
                    load_b1(1, nc.scalar)
                    load_w1(1, 0, nc.scalar)
                    load_w1(1, 1, nc.scalar)
                    load_b2(1, nc.scalar)
                    load_w2(1, nc.scalar)

                # layer 1: h = silu(W1 @ x + b1), h[p, m*L + l] fp16
                hsb = hpool.tile([P, MT * L], _FP16, tag="h")
                for m in range(MT):
                    psh = pspool.tile([P, L], _FP32, tag="ps")
                    for k in range(KT):
                        for n in range(NT):
                            nc.tensor.matmul(
                                psh[:, n * 512:(n + 1) * 512],
                                w1sb[e][k][:, m * P:(m + 1) * P],
                                xsb[:, k * L + n * 512:k * L + (n + 1) * 512],
                                start=(k == 0),
                                stop=(k == KT - 1),
                            )
                    nc.scalar.activation(
                        hsb[:, m * L:(m + 1) * L],
                        psh[:],
                        mybir.ActivationFunctionType.Silu,
                        bias=b1sb[:, e * MT + m: e * MT + m + 1],
                    )

                # layer 2: y = W2 @ h + b2, fp16 out
                last_pair = (t == len(pairs) - 1)
                if last_pair:
                    # n-outer with a separate 1-bank psum tile per (j, n):
                    # DVE(n0) overlaps the n1 matmul chain; per-[128,512]
                    # stores shorten the drain after the final matmul
                    for j in range(JT):
                        for n in range(NT):
                            psn = pspool.tile([P, 512], _FP32, tag="ps",
                                              name=f"psn_{j}_{n}")
                            for q in range(QT):
                                nc.tensor.matmul(
                                    psn[:],
                                    w2sb[e][:, q * C + j * P:
                                            q * C + (j + 1) * P],
                                    hsb[:, q * L + n * 512:
                                          q * L + (n + 1) * 512],
                                    start=(q == 0),
                                    stop=(q == QT - 1),
                                )
                            ysn = ypool.tile([P, 512], _FP16, tag="yt",
                                             name=f"ysn_{j}_{n}")
                            nc.vector.tensor_scalar_add(
                                ysn[:],
                                psn[:],
                                b2sb[:, e * JT + j: e * JT + j + 1],
                            )
                            # alternate DGE queues so the 4 drain stores
                            # transfer pairwise-parallel
                            eng = nc.sync if (j * NT + n) % 2 == 0 else nc.scalar
                            eng.dma_start(
                                ys_d.ap()[b,
                                          e * C + j * P: e * C + (j + 1) * P,
                                          n * 512:(n + 1) * 512],
                                ysn[:],
                            )
                    continue

                ysb = ypool.tile([P, JT * L], _FP16, tag="y",
                                 name=f"ysb_{e}_{b}")
                for j in range(JT):
                    psy = pspool.tile([P, L], _FP32, tag="ps")
                    for q in range(QT):
                        for n in range(NT):
                            nc.tensor.matmul(
                                psy[:, n * 512:(n + 1) * 512],
                                w2sb[e][:, q * C + j * P: q * C + (j + 1) * P],
                                hsb[:, q * L + n * 512: q * L + (n + 1) * 512],
                                start=(q == 0),
                                stop=(q == QT - 1),
                            )
                    nc.vector.tensor_scalar_add(
                        ysb[:, j * L:(j + 1) * L],
                        psy[:],
                        b2sb[:, e * JT + j: e * JT + j + 1],
                    )
                nc.sync.dma_start(
                    ys_d.ap()[b, e * C:(e + 1) * C, :]
                    .rearrange("(j p) l -> p j l", p=P),
                    ysb[:].rearrange("p (j l) -> p j l", j=JT),
                )

    nc.compile()
    return nc


_NC_CACHE = None


def _get_nc():
    global _NC_CACHE
    if _NC_CACHE is None:
        _NC_CACHE = _build()
    return _NC_CACHE


def _shard_inputs(x, W1, b1, W2, b2):
    """Full fp32 inputs -> list of 8 per-core fp16-packed input dicts."""
    x16 = np.ascontiguousarray(x, dtype=np.float32).astype(np.float16)
    # lhsT layouts (fp16):
    # w1p[(e k) p, f] = W1r[e, f, k*128+p]   (W1r = W1.reshape(E, F, C))
    # w2p[(e p), q*C+c] = W2r[e, c, q*128+p] (W2r = W2.reshape(E, C, F))
    w1t = W1.astype(np.float32).reshape(E, F, C).transpose(0, 2, 1)
    w1p = np.ascontiguousarray(w1t).astype(np.float16).reshape(E * KT * P, F)
    w2t = W2.astype(np.float32).reshape(E, C, F).transpose(0, 2, 1)  # [E,F,C]
    w2p = (np.ascontiguousarray(w2t).astype(np.float16)
           .reshape(E, QT, P, C).transpose(0, 2, 1, 3)
           .reshape(E, P, QT * C))
    w2p = np.ascontiguousarray(w2p).reshape(E * P, QT * C)
    b1p = np.ascontiguousarray(
        b1.astype(np.float32).reshape(E, MT, P).transpose(0, 2, 1)
    ).reshape(E * P, MT)
    b2p = np.ascontiguousarray(
        b2.astype(np.float32).reshape(E, JT, P).transpose(0, 2, 1)
    ).reshape(E * P, JT)
    in_maps = []
    for i in range(NCORES):
        in_maps.append({
            "xs": np.ascontiguousarray(x16[:, i * EPC * C:(i + 1) * EPC * C, :]),
            "w1p": np.ascontiguousarray(
                w1p[i * EPC * KT * P:(i + 1) * EPC * KT * P]),
            "b1p": np.ascontiguousarray(b1p[i * EPC * P:(i + 1) * EPC * P]),
            "w2p": np.ascontiguousarray(w2p[i * EPC * P:(i + 1) * EPC * P]),
            "b2p": np.ascontiguousarray(b2p[i * EPC * P:(i + 1) * EPC * P]),
        })
    return in_maps


def run(x, W1, b1, W2, b2, trace=False, **trace_kwargs):
    nc = _get_nc()
    in_maps = _shard_inputs(x, W1, b1, W2, b2)
    res = run_bass_kernel_spmd(
        nc, in_maps, core_ids=list(range(NCORES)), trace=trace, **trace_kwargs
    )
    y = np.concatenate(
        [res.results[i]["ys"].astype(np.float32) for i in range(NCORES)],
        axis=1,
    )
    return y, res


def kernel(x, W1, b1, W2, b2):
    y, _ = run(x, W1, b1, W2, b2)
    return y.astype(np.float32)


# revision 15
# speedup vs baseline: 1.0036x; 1.0036x over previous
"""Expert-parallel grouped-MLP (MoE experts) kernel for 8 Trainium2 cores.

Problem: y = W2_e @ silu(W1_e @ x_e + b1_e) + b2_e for E=16 independent
experts (grouped 1x1 conv), B=8 batches, C=256 channels/expert, CAP=4,
L=1024 positions.

Sharding: expert-parallel — core i owns experts {2i, 2i+1}; no cross-core
communication. All operands are cast/packed to fp16 lhsT layouts on the
HOST, so the device does zero dtype conversion and every load is a single
contiguous DMA:

  per (b, e) pair:  1 DMA x[128, 2048] fp16 (both k-tiles merged)
    layer 1: 8 m-tiles x (2k x 2n) fp16 matmuls -> PSUM[128,1024] fp32
             ACT silu(. + b1) PSUM -> h SBUF [128, 8x1024] fp16
    layer 2: 2 j-tiles x (8q x 2n) accumulating matmuls -> PSUM[128,1024]
             DVE + b2 PSUM -> y SBUF fp16, 1 merged DMA out per pair

Output is fp16 on device, upcast to fp32 on the host. PE floor is
1024 matmuls x 512 cols @ 2.4GHz = 218.4us/core; startup (fp16 weights,
no casts) and tail (fewer DMAs/semaphores) are trimmed vs the fp32-staged
version. Warmup matmuls cover the PE p-state ramp during initial DMAs.
"""
import numpy as np

import concourse.tile as tile
from concourse import bacc, mybir
from concourse.bass_utils import run_bass_kernel_spmd

# Problem constants (hardcoded per contract)
B, E, C, CAP, L = 8, 16, 256, 4, 1024
F = C * CAP            # 1024 hidden per expert
NCORES = 8
EPC = E // NCORES      # 2 experts per core
P = 128                # partitions
KT = C // P            # 2 k-tiles (layer-1 contraction)
MT = F // P            # 8 m-tiles (layer-1 output partitions)
JT = C // P            # 2 j-tiles (layer-2 output partitions)
QT = F // P            # 8 q-tiles (layer-2 contraction)
NT = L // 512          # 2 n-tiles of 512 cols
N_WARMUP = 8           # dummy PE warmup matmuls: cover the p-state ramp AND
                       # bridge startup-DMA jitter — a PE idle gap here resets
                       # the ramp and costs ~1.5us of mid-p-state real matmuls

_FP32 = mybir.dt.float32
_FP16 = mybir.dt.float16


def _build():
    nc = bacc.Bacc("TRN2", target_bir_lowering=False, debug=False)

    xs_d = nc.dram_tensor("xs", [B, EPC * C, L], _FP16, kind="ExternalInput")
    w1p_d = nc.dram_tensor("w1p", [EPC * KT * P, F], _FP16, kind="ExternalInput")
    b1p_d = nc.dram_tensor("b1p", [EPC * P, MT], _FP32, kind="ExternalInput")
    w2p_d = nc.dram_tensor("w2p", [EPC * P, QT * C], _FP16, kind="ExternalInput")
    b2p_d = nc.dram_tensor("b2p", [EPC * P, JT], _FP32, kind="ExternalInput")
    ys_d = nc.dram_tensor("ys", [B, EPC * C, L], _FP16, kind="ExternalOutput")

    with tile.TileContext(nc) as tc:
        with (
            tc.tile_pool(name="const", bufs=1) as cpool,
            tc.tile_pool(name="x", bufs=3) as xpool,
            tc.tile_pool(name="h", bufs=2) as hpool,
            tc.tile_pool(name="y", bufs=2) as ypool,
            tc.tile_pool(name="ps", bufs=4, space="PSUM") as pspool,
        ):
            # ---- PE warmup: zero bf16 matmuls with no DMA deps ----
            wdum = cpool.tile([P, P], mybir.dt.bfloat16, tag="wdum")
            rdum = cpool.tile([P, 512], mybir.dt.bfloat16, tag="rdum")
            # memset on Pool: its post-barrier slot opens earliest, so the
            # first warmup matmul issues ~0.6us sooner than via DVE
            nc.gpsimd.memset(wdum[:], 0.0)
            nc.gpsimd.memset(rdum[:], 0.0)
            for i in range(N_WARMUP):
                pdum = pspool.tile([P, L], _FP32, tag="ps")
                nc.tensor.matmul(pdum[:, :512], wdum[:], rdum[:],
                                 start=True, stop=True)

            # ---- weight/bias tiles: host-packed fp16, single DMAs ----
            # w1sb[e][k]: [128, F];       [p, f] = W1r[ge, f, k*128+p]
            # w2sb[e]:    [128, QT*C];    [p, q*C+c] = W2r[ge, c, q*128+p]
            w1sb = [[cpool.tile([P, F], _FP16, tag=f"w1_{e}_{k}",
                                name=f"w1sb_{e}_{k}")
                     for k in range(KT)] for e in range(EPC)]
            w2sb = [cpool.tile([P, QT * C], _FP16, tag=f"w2_{e}",
                               name=f"w2sb_{e}")
                    for e in range(EPC)]
            b1sb = cpool.tile([P, EPC * MT], _FP32, tag="b1")  # col e*MT+m
            b2sb = cpool.tile([P, EPC * JT], _FP32, tag="b2")  # col e*JT+j

            # weight/bias loads go on the Activation engine's HW-DGE so
            # their transfers run on a separate hardware queue from the
            # x/y stream on the SP queue (each DGE serializes its queue)
            def load_w1(e, k, eng):
                eng.dma_start(
                    w1sb[e][k][:],
                    w1p_d.ap()[(e * KT + k) * P:(e * KT + k + 1) * P, :],
                )

            def load_w2(e, eng):
                eng.dma_start(
                    w2sb[e][:],
                    w2p_d.ap()[e * P:(e + 1) * P, :],
                )

            def load_b1(e, eng):
                eng.dma_start(
                    b1sb[:, e * MT:(e + 1) * MT],
                    b1p_d.ap()[e * P:(e + 1) * P, :],
                )

            def load_b2(e, eng):
                eng.dma_start(
                    b2sb[:, e * JT:(e + 1) * JT],
                    b2p_d.ap()[e * P:(e + 1) * P, :],
                )

            def load_x(b, e, split=False):
                # one tile [128, KT*L] fp16; col k*L+l = x[b, e*C+k*128+p, l]
                xt = xpool.tile([P, KT * L], _FP16, tag="x", name=f"x_{b}_{e}")
                if split:
                    # first pair: per-k DMAs on BOTH DGE queues so the two
                    # halves transfer in parallel with the w1 k-tiles
                    for k, eng in ((0, nc.sync), (1, nc.scalar)):
                        eng.dma_start(
                            xt[:, k * L:(k + 1) * L],
                            xs_d.ap()[b, e * C + k * P:e * C + (k + 1) * P, :],
                        )
                else:
                    nc.sync.dma_start(
                        xt[:].rearrange("p (k l) -> p k l", k=KT),
                        xs_d.ap()[b, e * C:(e + 1) * C, :]
                        .rearrange("(k p) l -> p k l", p=P),
                    )
                return xt

            # startup-critical order, two parallel queues:
            #   SP DGE:  w1(0,0), x0[k0], x(pair1), ...
            #   ACT DGE: w1(0,1), x0[k1], b1, b2, w2 (needed ~20us in)
            # expert-1 weights are deferred into the pair loop (needed
            # only at pair 8).
            load_w1(0, 0, nc.sync)
            load_w1(0, 1, nc.scalar)
            x0 = load_x(0, 0, split=True)
            load_b1(0, nc.scalar)
            load_b2(0, nc.scalar)
            load_w2(0, nc.scalar)
            # actdum AFTER the ACT-DGE triggers: the lazy ACT_TABLE_LOAD
            # (~1.3us) it induces must not delay the startup weight DMAs
            actdum = cpool.tile([P, 1], _FP32, tag="actdum")
            nc.scalar.activation(actdum[:], rdum[:, :1],
                                 mybir.ActivationFunctionType.Silu, bias=0.0)

            pairs = [(e, b) for e in range(EPC) for b in range(B)]
            xtiles = {(0, 0): x0}

            def ensure_x(t2):
                if 0 <= t2 < len(pairs) and pairs[t2] not in xtiles:
                    pe, pb = pairs[t2]
                    xtiles[pairs[t2]] = load_x(pb, pe)

            # ---- per-(expert, batch) pipeline ----
            for t, (e, b) in enumerate(pairs):
                xsb = xtiles.pop((e, b))
                if t == 1:
                    # early prefetches in flight; queue expert-1 weights on
                    # the ACT DGE (needed at pair 8, ~120us in)
                    load_b1(1, nc.scalar)
                    load_w1(1, 0, nc.scalar)
                    load_w1(1, 1, nc.scalar)
                    load_b2(1, nc.scalar)
                    load_w2(1, nc.scalar)

                # layer 1: h = silu(W1 @ x + b1), h[p, m*L + l] fp16
                hsb = hpool.tile([P, MT * L], _FP16, tag="h")
                for m in range(MT):
                    psh = pspool.tile([P, L], _FP32, tag="ps")
                    for k in range(KT):
                        for n in range(NT):
                            nc.tensor.matmul(
                                psh[:, n * 512:(n + 1) * 512],
                                w1sb[e][k][:, m * P:(m + 1) * P],
                                xsb[:, k * L + n * 512:k * L + (n + 1) * 512],
                                start=(k == 0),
                                stop=(k == KT - 1),
                            )
                    nc.scalar.activation(
                        hsb[:, m * L:(m + 1) * L],
                        psh[:],
                        mybir.ActivationFunctionType.Silu,
                        bias=b1sb[:, e * MT + m: e * MT + m + 1],
                    )

                # x prefetch triggers emitted AFTER layer 1 so their
                # transfers stay out of the startup-critical DMA window
                # (needed only ~2 pairs = ~27us later)
                ensure_x(t + 1)
                ensure_x(t + 2)

                # layer 2: y = W2 @ h + b2, fp16 out
                last_pair = (t == len(pairs) - 1)
                if last_pair:
                    # n-outer with a separate 1-bank psum tile per (j, n):
                    # DVE(n0) overlaps the n1 matmul chain; per-[128,512]
                    # stores shorten the drain after the final matmul
                    for j in range(JT):
                        for n in range(NT):
                            psn = pspool.tile([P, 512], _FP32, tag="ps",
                                              name=f"psn_{j}_{n}")
                            for q in range(QT):
                                nc.tensor.matmul(
                                    psn[:],
                                    w2sb[e][:, q * C + j * P:
                                            q * C + (j + 1) * P],
                                    hsb[:, q * L + n * 512:
                                          q * L + (n + 1) * 512],
                                    start=(q == 0),
                                    stop=(q == QT - 1),
                                )
                            ysn = ypool.tile([P, 512], _FP16, tag="yt",
                                             name=f"ysn_{j}_{n}")
                            nc.vector.tensor_scalar_add(
                                ysn[:],
                                psn[:],
                                b2sb[:, e * JT + j: e * JT + j + 1],
                            )
                            # alternate DGE queues so the 4 drain stores
                            # transfer pairwise-parallel
                            eng = nc.sync if (j * NT + n) % 2 == 0 else nc.scalar
                            eng.dma_start(
                                ys_d.ap()[b,
                                          e * C + j * P: e * C + (j + 1) * P,
                                          n * 512:(n + 1) * 512],
                                ysn[:],
                            )
                    continue

                ysb = ypool.tile([P, JT * L], _FP16, tag="y",
                                 name=f"ysb_{e}_{b}")
                for j in range(JT):
                    psy = pspool.tile([P, L], _FP32, tag="ps")
                    for q in range(QT):
                        for n in range(NT):
                            nc.tensor.matmul(
                                psy[:, n * 512:(n + 1) * 512],
                                w2sb[e][:, q * C + j * P: q * C + (j + 1) * P],
                                hsb[:, q * L + n * 512: q * L + (n + 1) * 512],
                                start=(q == 0),
                                stop=(q == QT - 1),
                            )
                    nc.vector.tensor_scalar_add(
                        ysb[:, j * L:(j + 1) * L],
                        psy[:],
                        b2sb[:, e * JT + j: e * JT + j + 1],
                    )
                nc.sync.dma_start(
                    ys_d.ap()[b, e * C:(e + 1) * C, :]
                    .rearrange("(j p) l -> p j l", p=P),
                    ysb[:].rearrange("p (j l) -> p j l", j=JT),
                )

    nc.compile()
    return nc


_NC_CACHE = None


def _get_nc():
    global _NC_CACHE
    if _NC_CACHE is None:
        _NC_CACHE = _build()
    return _NC_CACHE


def _shard_inputs(x, W1, b1, W2, b2):
    """Full fp32 inputs -> list of 8 per-core fp16-packed input dicts."""
    x16 = np.ascontiguousarray(x, dtype=np.float32).astype(np.float16)
    # lhsT layouts (fp16):
    # w1p[(e k) p, f] = W1r[e, f, k*128+p]   (W1r = W1.reshape(E, F, C))
    # w2p[(e p), q*C+c] = W2r[e, c, q*128+p] (W2r = W2.reshape(E, C, F))
    w1t = W1.astype(np.float32).reshape(E, F, C).transpose(0, 2, 1)
    w1p = np.ascontiguousarray(w1t).astype(np.float16).reshape(E * KT * P, F)
    w2t = W2.astype(np.float32).reshape(E, C, F).transpose(0, 2, 1)  # [E,F,C]
    w2p = (np.ascontiguousarray(w2t).astype(np.float16)
           .reshape(E, QT, P, C).transpose(0, 2, 1, 3)
           .reshape(E, P, QT * C))
    w2p = np.ascontiguousarray(w2p).reshape(E * P, QT * C)
    b1p = np.ascontiguousarray(
        b1.astype(np.float32).reshape(E, MT, P).transpose(0, 2, 1)
    ).reshape(E * P, MT)
    b2p = np.ascontiguousarray(
        b2.astype(np.float32).reshape(E, JT, P).transpose(0, 2, 1)
    ).reshape(E * P, JT)
    in_maps = []
    for i in range(NCORES):
        in_maps.append({
            "xs": np.ascontiguousarray(x16[:, i * EPC * C:(i + 1) * EPC * C, :]),
            "w1p": np.ascontiguousarray(
                w1p[i * EPC * KT * P:(i + 1) * EPC * KT * P]),
            "b1p": np.ascontiguousarray(b1p[i * EPC * P:(i + 1) * EPC * P]),
            "w2p": np.ascontiguousarray(w2p[i * EPC * P:(i + 1) * EPC * P]),
            "b2p": np.ascontiguousarray(b2p[i * EPC * P:(i + 1) * EPC * P]),
        })
    return in_maps


def run(x, W1, b1, W2, b2, trace=False, **trace_kwargs):
    nc = _get_nc()
    in_maps = _shard_inputs(x, W1, b1, W2, b2)
    res = run_bass_kernel_spmd(
        nc, in_maps, core_ids=list(range(NCORES)), trace=trace, **trace_kwargs
    )
    y = np.concatenate(
        [res.results[i]["ys"].astype(np.float32) for i in range(NCORES)],
        axis=1,
    )
    return y, res


def kernel(x, W1, b1, W2, b2):
    y, _ = run(x, W1, b1, W2, b2)
    return y.astype(np.float32)
